# revision 19
# baseline (speedup 1.0000x reference)
"""Trainium2 Bass kernel for nn_BasicMoe (N=4096, D=1024, E=8, DFF=2048, top-2).

Contract: kernel(norm_data, gate_w, w1, w2) -> np.ndarray [4096, 1024] fp32.
Sharding: data-parallel over tokens — each of the 8 NeuronCores takes a
512-token shard and runs the full MoE for it on-device (gate + routing +
experts + weighted combine); the host only slices/transposes/casts inputs and
concatenates the 8 disjoint output shards. No collectives needed.

Top-2 sparsity is exploited on-device: each expert processes only the tokens
routed to it (capacity C=192 of 512; measured per-(core,expert) max is 153
for the reference's fixed seed, and approximately 128 + 6.5 sigma for any
seed). The gather/scatter runs on the PE via 0/1 selection matrices built
on-device from the gate:

  pos[e, tok]  = cumsum(keep[e, tok]) along tokens (DVE prefix scan)
  S[tok, j]    = 1 iff token tok is the j-th routed token of expert e
  STW[j, tok]  = (S * gate_weight)^T via PE transposes
  xg  = x^T @ S              (gather of fp16 activations, exact)
  hT  = silu(w1_e^T @ xg)    (fp16 matmul, fp32 PSUM, SiLU on ScalarE)
  y   = hT^T @ w2_e          (fp16 matmul, fp32 PSUM)
  acc += STW^T @ y           (weighted scatter-add back to token order)

The gate runs in fp32 so top-2 selection matches the reference exactly.
All big matmuls use fp16 (same PE speed as bf16, 8x finer mantissa).
"""

import os
import sys

if "/opt/trn_rl_repo" not in sys.path:
    sys.path.insert(0, "/opt/trn_rl_repo")

import numpy as np

import concourse.bass as bass
import concourse.mybir as mybir
import concourse.tile as tile
from concourse import bacc
from concourse.bass import ts
from concourse.bass_utils import run_bass_kernel_spmd
from concourse.masks import make_identity

P = 128
N_TOK = 4096
D = 1024
E = 8
DFF = 2048
N_CORES = 8
TOK = N_TOK // N_CORES  # 512 tokens per core
KC_D = D // P           # 8 contraction chunks over D
KC_F = DFF // P         # 16 contraction chunks over DFF
NT = TOK // P           # 4 token tiles per core
FREE = 512
ND = D // FREE          # 2 output halves
C = 192                 # per-expert token capacity
JT = [(0, 128), (128, 64)]  # j-tiles of the capacity dim: (start, size)

F32 = mybir.dt.float32
F16 = mybir.dt.float16
AF = mybir.ActivationFunctionType
ALU = mybir.AluOpType
AX = mybir.AxisListType

_CACHE = {}
LAST_RESULT = None


def _build():
    nc = bacc.Bacc(trn_type="TRN2", debug=False, num_devices=N_CORES)

    xT_d = nc.dram_tensor("xT", [D, TOK], F32, kind="ExternalInput").ap()
    xh_d = nc.dram_tensor("xh", [TOK, D], F16, kind="ExternalInput").ap()
    gwT_d = nc.dram_tensor("gwT", [D, E], F32, kind="ExternalInput").ap()
    w1_d = nc.dram_tensor("w1", [E, D, DFF], F16, kind="ExternalInput").ap()
    w2_d = nc.dram_tensor("w2", [E, DFF, D], F16, kind="ExternalInput").ap()
    out_d = nc.dram_tensor("out", [TOK, D], F32, kind="ExternalOutput").ap()
    out_r = out_d.rearrange("(t p) d -> p t d", p=P)

    with tile.TileContext(nc) as tc:
        with (
            tc.tile_pool(name="const", bufs=1) as cpool,
            tc.tile_pool(name="gate", bufs=2) as gpool,
            tc.tile_pool(name="w1p", bufs=2) as w1_pool,
            tc.tile_pool(name="w2p", bufs=2) as w2_pool,
            tc.tile_pool(name="sel", bufs=2) as sel_pool,
            tc.tile_pool(name="htp", bufs=1) as ht_pool,
            tc.tile_pool(name="ysb", bufs=1) as y_pool,
            tc.tile_pool(name="psh", bufs=2, space="PSUM") as psum_h,
            tc.tile_pool(name="psy", bufs=2, space="PSUM") as psum_y,
            tc.tile_pool(name="pss", bufs=2, space="PSUM") as psum_s,
            tc.tile_pool(name="pst", bufs=2, space="PSUM") as psum_t,
        ):
            # ---- resident inputs & constants ----
            # xT feeds the gate, the critical path at kernel start: split the
            # DMA per contraction chunk so it spreads across DMA queues.
            gwT = cpool.tile([P, KC_D, E], F32, tag="gwT")
            nc.sync.dma_start(gwT[:], gwT_d.rearrange("(kc p) e -> p kc e", p=P))
            xT = cpool.tile([P, KC_D, TOK], F32, tag="xT")
            xT_r = xT_d.rearrange("(kc p) t -> p kc t", p=P)
            for kc in range(KC_D):
                nc.sync.dma_start(xT[:, kc, :], xT_r[:, kc, :])
            xh = cpool.tile([P, NT, D], F16, tag="xh")
            nc.sync.dma_start(xh[:], xh_d.rearrange("(t p) d -> p t d", p=P))

            # prefetch expert-0 weights ahead of the gate computation
            w1_tiles = {}
            w2_tiles = {}

            def fetch_weights(e):
                w1t = w1_pool.tile([P, KC_D, DFF], F16, tag="w1t", name="w1t")
                w1_r = w1_d[e].rearrange("(kc p) f -> p kc f", p=P)
                for h in range(2):
                    nc.sync.dma_start(
                        w1t[:, ts(h, KC_D // 2), :], w1_r[:, ts(h, KC_D // 2), :]
                    )
                w2t = w2_pool.tile([P, KC_F, D], F16, tag="w2t", name="w2t")
                w2_r = w2_d[e].rearrange("(kc p) d -> p kc d", p=P)
                for h in range(2):
                    nc.sync.dma_start(
                        w2t[:, ts(h, KC_F // 2), :], w2_r[:, ts(h, KC_F // 2), :]
                    )
                w1_tiles[e] = w1t
                w2_tiles[e] = w2t

            fetch_weights(0)

            # Warm the PE's HAM clock gate with dummy f16 matmuls while the
            # input DMAs land, so the gate matmuls run at 2.4 GHz instead of
            # the cold 1.2 GHz (transpose-mode doesn't count as PE-busy).
            # The operand comes from a DVE memset, ready ~3us before the
            # gpsimd-built identities.
            warm = cpool.tile([P, P], F16, tag="warm")
            nc.vector.memset(warm[:], 1.0)
            pwarm_full = psum_t.tile([P, P], F16, tag="pst", name="pwarm")
            pwarm = pwarm_full.bitcast(F32)[:, : P // 2]
            for i in range(40):
                nc.tensor.matmul(
                    pwarm[:], warm[:], warm[:, :64], start=(i == 0),
                    stop=(i == 39),
                )

            id128h = cpool.tile([P, P], F16, tag="id128h")
            make_identity(nc, id128h[:])
            id128 = cpool.tile([P, P], F32, tag="id128")
            make_identity(nc, id128[:])
            id8 = cpool.tile([E, E], F32, tag="id8")
            make_identity(nc, id8[:])
            # iota_c[p, j] = j
            iota_c = cpool.tile([P, C], F32, tag="iota_c")
            nc.gpsimd.iota(
                iota_c[:], pattern=[[1, C]], base=0, channel_multiplier=0,
                allow_small_or_imprecise_dtypes=True,
            )

            # ---- gate: Wt[tok, e] fp32 weights + keep mask ----
            # logitsT[e, tok] accumulates one matmul per xT DMA chunk, so the
            # gate pipelines behind the input transfer; tiny 8-column weight
            # loads instead of 128-column fp32 ones.
            Wt = cpool.tile([P, NT, E], F32, tag="Wt")
            Invt = cpool.tile([P, NT], F32, tag="Invt")
            keep = cpool.tile([P, NT, E], F32, tag="keep")
            keepT = cpool.tile([E, TOK], F32, tag="keepT")
            plog_full = psum_s.tile([P, FREE], F32, tag="pss", name="plog")
            plog = plog_full[:E, :TOK]
            for kc in range(KC_D):
                nc.tensor.matmul(
                    plog[:],
                    gwT[:, kc, :],
                    xT[:, kc, :],
                    start=(kc == 0),
                    stop=(kc == KC_D - 1),
                )
            logT = cpool.tile([E, TOK], F32, tag="logT")
            nc.scalar.copy(logT[:], plog[:])
            for t in range(NT):
                pg_full = psum_h.tile([P, FREE], F32, tag="ph", name="pg")
                pg = pg_full[:, :E]
                nc.tensor.transpose(pg[:], logT[:, ts(t, P)], id8[:])
                L = gpool.tile([P, E], F32, tag="L")
                nc.vector.tensor_copy(L[:], pg[:])
                m1 = gpool.tile([P, 1], F32, tag="m1")
                nc.vector.reduce_max(m1[:], L[:], axis=AX.X)
                L2 = gpool.tile([P, E], F32, tag="L2")
                ismax = gpool.tile([P, E], F32, tag="ismax")
                nc.vector.tensor_tensor(
                    ismax[:], L[:], m1.to_broadcast([P, E]), ALU.is_ge
                )
                nc.vector.scalar_tensor_tensor(
                    L2[:], ismax[:], -1.0e30, L[:], ALU.mult, ALU.add
                )
                m2 = gpool.tile([P, 1], F32, tag="m2")
                nc.vector.reduce_max(m2[:], L2[:], axis=AX.X)
                nc.vector.tensor_tensor(
                    keep[:, t, :], L[:], m2.to_broadcast([P, E]), ALU.is_ge
                )
                ptk_full = psum_s.tile([P, FREE], F32, tag="pss", name="ptk")
                ptk = ptk_full[:E, :P]
                nc.tensor.transpose(ptk[:], keep[:, t, :], id128[:])
                nc.scalar.copy(keepT[:, ts(t, P)], ptk[:])
                negm1 = gpool.tile([P, 1], F32, tag="negm1")
                nc.vector.tensor_scalar_mul(negm1[:], m1[:], -1.0)
                expw = gpool.tile([P, E], F32, tag="expw")
                nc.scalar.activation(expw[:], L[:], AF.Exp, bias=negm1[:, 0:1])
                nc.vector.tensor_mul(Wt[:, t, :], expw[:], keep[:, t, :])
                den = gpool.tile([P, 1], F32, tag="den")
                nc.vector.reduce_sum(den[:], Wt[:, t, :], axis=AX.X)
                nc.vector.reciprocal(Invt[:, t : t + 1], den[:])

            # ---- routing positions ----
            # pos[e, tok] = inclusive cumsum of keepT along tokens
            pos = cpool.tile([E, TOK], F32, tag="pos")
            nc.vector.tensor_tensor_scan(
                pos[:], keepT[:], keepT[:], 0.0, ALU.add, ALU.bypass
            )
            # posm1m = pos*keep - 1: slot index of routed tokens, -1 otherwise
            posm1m = cpool.tile([E, TOK], F32, tag="posm1m")
            nc.vector.tensor_tensor(posm1m[:], pos[:], keepT[:], ALU.mult)
            nc.vector.tensor_scalar_add(posm1m[:], posm1m[:], -1.0)
            # posm1mT[tok, e] for building S with per-partition compares
            posm1mT = cpool.tile([P, NT, E], F32, tag="posm1mT")
            for t in range(NT):
                pt2_full = psum_s.tile([P, FREE], F32, tag="pss", name="ptp")
                ptp = pt2_full[:, :E]
                nc.tensor.transpose(ptp[:], posm1m[:, ts(t, P)], id8[:])
                nc.scalar.copy(posm1mT[:, t, :], ptp[:])

            acc = cpool.tile([P, NT, D], F32, tag="acc")
            nc.vector.memset(acc[:], 0.0)

            # S[tok, j] selection matrix, fp16 {0,1}; SW = S scaled by the
            # per-token gate weight (so the scatter applies the combine
            # weight for free). Built one expert ahead so the in-order DVE
            # queue doesn't serialize them behind the previous expert's
            # epilogue.
            S_tiles = {}
            _CACHE_XG = {}

            def build_sel(pair):
                # S for experts (2*pair, 2*pair+1) side by side: one gather
                # matmul pass with a 384-wide moving operand serves both.
                S = sel_pool.tile([P, NT, 2 * C], F16, tag="S", name="S")
                for sub in range(2):
                    e = 2 * pair + sub
                    for t in range(NT):
                        nc.vector.tensor_scalar(
                            S[:, t, sub * C : (sub + 1) * C],
                            iota_c[:],
                            posm1mT[:, t, e : e + 1],
                            None,
                            ALU.is_equal,
                        )
                S_tiles[pair] = S

            build_sel(0)

            # ---- expert loop (processed in pairs sharing one gather) ----
            for e in range(E):
                pair, sub = divmod(e, 2)
                if e + 1 < E:
                    fetch_weights(e + 1)
                if sub == 0 and pair + 1 < E // 2:
                    build_sel(pair + 1)
                w1t = w1_tiles.pop(e)
                w2t = w2_tiles.pop(e)
                Spair = S_tiles[pair]
                S = Spair[:, :, sub * C : (sub + 1) * C]

                if sub == 0:
                    # gather both experts of the pair: xg2[d, 0:C]=e, [C:2C]=e+1
                    xg2 = ht_pool.tile([P, KC_D, 2 * C], F16, tag="xg", name="xg2")
                    for kc in range(KC_D):
                        px = psum_h.tile([P, FREE], F32, tag="ph", name="px")[
                            :, : 2 * C
                        ]
                        for t in range(NT):
                            nc.tensor.matmul(
                                px[:],
                                xh[:, t, ts(kc, P)],
                                Spair[:, t, :],
                                start=(t == 0),
                                stop=(t == NT - 1),
                            )
                        nc.vector.tensor_copy(xg2[:, kc, :], px[:])
                    _CACHE_XG[pair] = xg2
                xg = _CACHE_XG[pair][:, :, sub * C : (sub + 1) * C]

                # SW = S scaled by the per-token gate weight; STW = (S*w)^T
                SW = ht_pool.tile([P, NT, C], F16, tag="SW", name="SW")
                for t in range(NT):
                    nc.vector.tensor_scalar(
                        SW[:, t, :],
                        S[:, t, :],
                        Wt[:, t, e : e + 1],
                        Invt[:, t : t + 1],
                        ALU.mult,
                        ALU.mult,
                    )
                STW = sel_pool.tile([P, 2, TOK], F16, tag="STW", name="STW")
                for jj, (js, jn) in enumerate(JT):
                    for t in range(NT):
                        pst_full = psum_t.tile([P, P], F16, tag="pst", name="pst")
                        pst = pst_full[:jn, :]
                        nc.tensor.transpose(
                            pst[:], SW[:, t, js : js + jn], id128h[:]
                        )
                        nc.scalar.copy(STW[:jn, jj, ts(t, P)], pst[:])

                # hT[f, j] = silu(w1_e^T @ xg)
                ht = ht_pool.tile([P, KC_F, C], F16, tag="ht")
                for m in range(KC_F):
                    ph = psum_h.tile([P, FREE], F32, tag="ph", name="ph")[:, :C]
                    for kc in range(KC_D):
                        nc.tensor.matmul(
                            ph[:],
                            w1t[:, kc, ts(m, P)],
                            xg[:, kc, :],
                            start=(kc == 0),
                            stop=(kc == KC_D - 1),
                        )
                    nc.scalar.activation(ht[:, m, :], ph[:], AF.Silu)

                # y[j, d] = hT^T @ w2_e -> fp16 (weights applied via STW)
                ysb = y_pool.tile([P, 2, ND, FREE], F16, tag="ysb")
                for jj, (js, jn) in enumerate(JT):
                    for dh in range(ND):
                        py = psum_y.tile([P, FREE], F32, tag="py", name="py")[
                            :jn, :
                        ]
                        for k in range(KC_F):
                            nc.tensor.matmul(
                                py[:],
                                ht[:, k, js : js + jn],
                                w2t[:, k, ts(dh, FREE)],
                                start=(k == 0),
                                stop=(k == KC_F - 1),
                            )
                        nc.vector.tensor_copy(ysb[:jn, jj, dh, :], py[:])

                # scatter-add: acc[tok, d] += (S*w)^T^T... i.e. STW^T @ y
                for t in range(NT):
                    for dh in range(ND):
                        ps = psum_s.tile([P, FREE], F32, tag="pss", name="ps")
                        for jj, (js, jn) in enumerate(JT):
                            nc.tensor.matmul(
                                ps[:],
                                STW[:jn, jj, ts(t, P)],
                                ysb[:jn, jj, dh, :],
                                start=(jj == 0),
                                stop=(jj == len(JT) - 1),
                            )
                        nc.vector.tensor_add(
                            acc[:, t, ts(dh, FREE)], ps[:], acc[:, t, ts(dh, FREE)]
                        )
                        if e == E - 1:
                            nc.sync.dma_start(
                                out_r[:, t, ts(dh, FREE)], acc[:, t, ts(dh, FREE)]
                            )

    nc.compile()
    return nc


def kernel(norm_data, gate_w, w1, w2):
    global LAST_RESULT
    if "nc" not in _CACHE:
        _CACHE["nc"] = _build()
    nc = _CACHE["nc"]

    x = np.ascontiguousarray(np.asarray(norm_data, dtype=np.float32))
    gwT = np.ascontiguousarray(np.asarray(gate_w, dtype=np.float32).T)
    w1b = np.ascontiguousarray(np.asarray(w1, dtype=np.float32)).astype(np.float16)
    w2b = np.ascontiguousarray(np.asarray(w2, dtype=np.float32)).astype(np.float16)

    in_maps = []
    for c in range(N_CORES):
        xc = x[c * TOK : (c + 1) * TOK]
        in_maps.append(
            {
                "xT": np.ascontiguousarray(xc.T),
                "xh": xc.astype(np.float16),
                "gwT": gwT,
                "w1": w1b,
                "w2": w2b,
            }
        )

    trace = os.environ.get("KERNEL_TRACE", "0") == "1"
    if trace:
        import antenv

        ext = os.environ.get("KERNEL_TRACE_HOOK_DIR", "/root/antenv_ext")
        if ext not in antenv.__path__:
            antenv.__path__.append(ext)
        from antenv.axon_hooks import set_axon_ntff_profile_hook
        from trn_agent_boot.trn_boot import _ntff_profile_via_ctypes

        set_axon_ntff_profile_hook(
            _ntff_profile_via_ctypes("/opt/axon/libaxon_pjrt.so")
        )

    res = run_bass_kernel_spmd(
        nc, in_maps, core_ids=list(range(N_CORES)), trace=trace
    )
    LAST_RESULT = res

    out = np.concatenate(
        [res.results[c]["out"] for c in range(N_CORES)], axis=0
    )
    return out


# revision 20
# speedup vs baseline: 1.0037x; 1.0037x over previous
"""Trainium2 Bass kernel for nn_BasicMoe (N=4096, D=1024, E=8, DFF=2048, top-2).

Contract: kernel(norm_data, gate_w, w1, w2) -> np.ndarray [4096, 1024] fp32.
Sharding: data-parallel over tokens — each of the 8 NeuronCores takes a
512-token shard and runs the full MoE for it on-device (gate + routing +
experts + weighted combine); the host only slices/transposes/casts inputs and
concatenates the 8 disjoint output shards. No collectives needed.

Top-2 sparsity is exploited on-device: each expert processes only the tokens
routed to it (capacity C=192 of 512; measured per-(core,expert) max is 153
for the reference's fixed seed, and approximately 128 + 6.5 sigma for any
seed). The gather/scatter runs on the PE via 0/1 selection matrices built
on-device from the gate:

  pos[e, tok]  = cumsum(keep[e, tok]) along tokens (DVE prefix scan)
  S[tok, j]    = 1 iff token tok is the j-th routed token of expert e
  STW[j, tok]  = (S * gate_weight)^T via PE transposes
  xg  = x^T @ S              (gather of fp16 activations, exact)
  hT  = silu(w1_e^T @ xg)    (fp16 matmul, fp32 PSUM, SiLU on ScalarE)
  y   = hT^T @ w2_e          (fp16 matmul, fp32 PSUM)
  acc += STW^T @ y           (weighted scatter-add back to token order)

The gate runs in fp32 so top-2 selection matches the reference exactly.
All big matmuls use fp16 (same PE speed as bf16, 8x finer mantissa).
"""

import os
import sys

if "/opt/trn_rl_repo" not in sys.path:
    sys.path.insert(0, "/opt/trn_rl_repo")

import numpy as np

import concourse.bass as bass
import concourse.mybir as mybir
import concourse.tile as tile
from concourse import bacc
from concourse.bass import ts
from concourse.bass_utils import run_bass_kernel_spmd
from concourse.masks import make_identity

P = 128
N_TOK = 4096
D = 1024
E = 8
DFF = 2048
N_CORES = 8
TOK = N_TOK // N_CORES  # 512 tokens per core
KC_D = D // P           # 8 contraction chunks over D
KC_F = DFF // P         # 16 contraction chunks over DFF
NT = TOK // P           # 4 token tiles per core
FREE = 512
ND = D // FREE          # 2 output halves
C = 192                 # per-expert token capacity
JT = [(0, 128), (128, 64)]  # j-tiles of the capacity dim: (start, size)

F32 = mybir.dt.float32
F16 = mybir.dt.float16
AF = mybir.ActivationFunctionType
ALU = mybir.AluOpType
AX = mybir.AxisListType

_CACHE = {}
LAST_RESULT = None


def _build():
    nc = bacc.Bacc(trn_type="TRN2", debug=False, num_devices=N_CORES)

    xT_d = nc.dram_tensor("xT", [D, TOK], F32, kind="ExternalInput").ap()
    xh_d = nc.dram_tensor("xh", [TOK, D], F16, kind="ExternalInput").ap()
    gwT_d = nc.dram_tensor("gwT", [D, E], F32, kind="ExternalInput").ap()
    w1_d = nc.dram_tensor("w1", [E, D, DFF], F16, kind="ExternalInput").ap()
    w2_d = nc.dram_tensor("w2", [E, DFF, D], F16, kind="ExternalInput").ap()
    out_d = nc.dram_tensor("out", [TOK, D], F32, kind="ExternalOutput").ap()
    out_r = out_d.rearrange("(t p) d -> p t d", p=P)

    with tile.TileContext(nc) as tc:
        with (
            tc.tile_pool(name="const", bufs=1) as cpool,
            tc.tile_pool(name="gate", bufs=2) as gpool,
            tc.tile_pool(name="w1p", bufs=2) as w1_pool,
            tc.tile_pool(name="w2p", bufs=2) as w2_pool,
            tc.tile_pool(name="sel", bufs=2) as sel_pool,
            tc.tile_pool(name="htp", bufs=1) as ht_pool,
            tc.tile_pool(name="ysb", bufs=1) as y_pool,
            tc.tile_pool(name="psh", bufs=2, space="PSUM") as psum_h,
            tc.tile_pool(name="psy", bufs=2, space="PSUM") as psum_y,
            tc.tile_pool(name="pss", bufs=2, space="PSUM") as psum_s,
            tc.tile_pool(name="pst", bufs=2, space="PSUM") as psum_t,
        ):
            # ---- resident inputs & constants ----
            # xT feeds the gate, the critical path at kernel start: split the
            # DMA per contraction chunk so it spreads across DMA queues.
            gwT = cpool.tile([P, KC_D, E], F32, tag="gwT")
            nc.sync.dma_start(gwT[:], gwT_d.rearrange("(kc p) e -> p kc e", p=P))
            xT = cpool.tile([P, KC_D, TOK], F32, tag="xT")
            xT_r = xT_d.rearrange("(kc p) t -> p kc t", p=P)
            for kc in range(KC_D):
                nc.sync.dma_start(xT[:, kc, :], xT_r[:, kc, :])
            xh = cpool.tile([P, NT, D], F16, tag="xh")
            nc.sync.dma_start(xh[:], xh_d.rearrange("(t p) d -> p t d", p=P))

            # prefetch expert-0 weights ahead of the gate computation
            w1_tiles = {}
            w2_tiles = {}

            def fetch_weights(e):
                w1t = w1_pool.tile([P, KC_D, DFF], F16, tag="w1t", name="w1t")
                w1_r = w1_d[e].rearrange("(kc p) f -> p kc f", p=P)
                for h in range(2):
                    nc.sync.dma_start(
                        w1t[:, ts(h, KC_D // 2), :], w1_r[:, ts(h, KC_D // 2), :]
                    )
                w2t = w2_pool.tile([P, KC_F, D], F16, tag="w2t", name="w2t")
                w2_r = w2_d[e].rearrange("(kc p) d -> p kc d", p=P)
                for h in range(2):
                    nc.sync.dma_start(
                        w2t[:, ts(h, KC_F // 2), :], w2_r[:, ts(h, KC_F // 2), :]
                    )
                w1_tiles[e] = w1t
                w2_tiles[e] = w2t

            fetch_weights(0)

            # Warm the PE's HAM clock gate with dummy f16 matmuls while the
            # input DMAs land, so the gate matmuls run at 2.4 GHz instead of
            # the cold 1.2 GHz (transpose-mode doesn't count as PE-busy).
            id128h = cpool.tile([P, P], F16, tag="id128h")
            make_identity(nc, id128h[:])
            pwarm_full = psum_t.tile([P, P], F16, tag="pst", name="pwarm")
            pwarm = pwarm_full.bitcast(F32)[:, : P // 2]
            for i in range(40):
                nc.tensor.matmul(
                    pwarm[:], id128h[:], id128h[:, :64], start=(i == 0),
                    stop=(i == 39),
                )
            id128 = cpool.tile([P, P], F32, tag="id128")
            make_identity(nc, id128[:])
            id8 = cpool.tile([E, E], F32, tag="id8")
            make_identity(nc, id8[:])
            # iota_c[p, j] = j
            iota_c = cpool.tile([P, C], F32, tag="iota_c")
            nc.gpsimd.iota(
                iota_c[:], pattern=[[1, C]], base=0, channel_multiplier=0,
                allow_small_or_imprecise_dtypes=True,
            )

            # ---- gate: Wt[tok, e] fp32 weights + keep mask ----
            # logitsT[e, tok] accumulates one matmul per xT DMA chunk, so the
            # gate pipelines behind the input transfer; tiny 8-column weight
            # loads instead of 128-column fp32 ones.
            Wt = cpool.tile([P, NT, E], F32, tag="Wt")
            Invt = cpool.tile([P, NT], F32, tag="Invt")
            keep = cpool.tile([P, NT, E], F32, tag="keep")
            keepT = cpool.tile([E, TOK], F32, tag="keepT")
            plog_full = psum_s.tile([P, FREE], F32, tag="pss", name="plog")
            plog = plog_full[:E, :TOK]
            for kc in range(KC_D):
                nc.tensor.matmul(
                    plog[:],
                    gwT[:, kc, :],
                    xT[:, kc, :],
                    start=(kc == 0),
                    stop=(kc == KC_D - 1),
                )
            logT = cpool.tile([E, TOK], F32, tag="logT")
            nc.scalar.copy(logT[:], plog[:])
            for t in range(NT):
                pg_full = psum_h.tile([P, FREE], F32, tag="ph", name="pg")
                pg = pg_full[:, :E]
                nc.tensor.transpose(pg[:], logT[:, ts(t, P)], id8[:])
                L = gpool.tile([P, E], F32, tag="L")
                nc.vector.tensor_copy(L[:], pg[:])
                m1 = gpool.tile([P, 1], F32, tag="m1")
                nc.vector.reduce_max(m1[:], L[:], axis=AX.X)
                L2 = gpool.tile([P, E], F32, tag="L2")
                ismax = gpool.tile([P, E], F32, tag="ismax")
                nc.vector.tensor_tensor(
                    ismax[:], L[:], m1.to_broadcast([P, E]), ALU.is_ge
                )
                nc.vector.scalar_tensor_tensor(
                    L2[:], ismax[:], -1.0e30, L[:], ALU.mult, ALU.add
                )
                m2 = gpool.tile([P, 1], F32, tag="m2")
                nc.vector.reduce_max(m2[:], L2[:], axis=AX.X)
                nc.vector.tensor_tensor(
                    keep[:, t, :], L[:], m2.to_broadcast([P, E]), ALU.is_ge
                )
                ptk_full = psum_s.tile([P, FREE], F32, tag="pss", name="ptk")
                ptk = ptk_full[:E, :P]
                nc.tensor.transpose(ptk[:], keep[:, t, :], id128[:])
                nc.scalar.copy(keepT[:, ts(t, P)], ptk[:])
                negm1 = gpool.tile([P, 1], F32, tag="negm1")
                nc.vector.tensor_scalar_mul(negm1[:], m1[:], -1.0)
                expw = gpool.tile([P, E], F32, tag="expw")
                nc.scalar.activation(expw[:], L[:], AF.Exp, bias=negm1[:, 0:1])
                nc.vector.tensor_mul(Wt[:, t, :], expw[:], keep[:, t, :])
                den = gpool.tile([P, 1], F32, tag="den")
                nc.vector.reduce_sum(den[:], Wt[:, t, :], axis=AX.X)
                nc.vector.reciprocal(Invt[:, t : t + 1], den[:])

            # ---- routing positions ----
            # pos[e, tok] = inclusive cumsum of keepT along tokens
            pos = cpool.tile([E, TOK], F32, tag="pos")
            nc.vector.tensor_tensor_scan(
                pos[:], keepT[:], keepT[:], 0.0, ALU.add, ALU.bypass
            )
            # posm1m = pos*keep - 1: slot index of routed tokens, -1 otherwise
            posm1m = cpool.tile([E, TOK], F32, tag="posm1m")
            nc.vector.tensor_tensor(posm1m[:], pos[:], keepT[:], ALU.mult)
            nc.vector.tensor_scalar_add(posm1m[:], posm1m[:], -1.0)
            # posm1mT[tok, e] for building S with per-partition compares
            posm1mT = cpool.tile([P, NT, E], F32, tag="posm1mT")
            for t in range(NT):
                pt2_full = psum_s.tile([P, FREE], F32, tag="pss", name="ptp")
                ptp = pt2_full[:, :E]
                nc.tensor.transpose(ptp[:], posm1m[:, ts(t, P)], id8[:])
                nc.scalar.copy(posm1mT[:, t, :], ptp[:])

            acc = cpool.tile([P, NT, D], F32, tag="acc")
            nc.vector.memset(acc[:], 0.0)

            # S[tok, j] selection matrix, fp16 {0,1}; SW = S scaled by the
            # per-token gate weight (so the scatter applies the combine
            # weight for free). Built one expert ahead so the in-order DVE
            # queue doesn't serialize them behind the previous expert's
            # epilogue.
            S_tiles = {}
            _CACHE_XG = {}

            def build_sel(pair):
                # S for experts (2*pair, 2*pair+1) side by side: one gather
                # matmul pass with a 384-wide moving operand serves both.
                S = sel_pool.tile([P, NT, 2 * C], F16, tag="S", name="S")
                for sub in range(2):
                    e = 2 * pair + sub
                    for t in range(NT):
                        nc.vector.tensor_scalar(
                            S[:, t, sub * C : (sub + 1) * C],
                            iota_c[:],
                            posm1mT[:, t, e : e + 1],
                            None,
                            ALU.is_equal,
                        )
                S_tiles[pair] = S

            build_sel(0)

            # ---- expert loop (processed in pairs sharing one gather) ----
            for e in range(E):
                pair, sub = divmod(e, 2)
                if e + 1 < E:
                    fetch_weights(e + 1)
                if sub == 0 and pair + 1 < E // 2:
                    build_sel(pair + 1)
                w1t = w1_tiles.pop(e)
                w2t = w2_tiles.pop(e)
                Spair = S_tiles[pair]
                S = Spair[:, :, sub * C : (sub + 1) * C]

                if sub == 0:
                    # gather both experts of the pair: xg2[d, 0:C]=e, [C:2C]=e+1
                    xg2 = ht_pool.tile([P, KC_D, 2 * C], F16, tag="xg", name="xg2")
                    for kc in range(KC_D):
                        px = psum_h.tile([P, FREE], F32, tag="ph", name="px")[
                            :, : 2 * C
                        ]
                        for t in range(NT):
                            nc.tensor.matmul(
                                px[:],
                                xh[:, t, ts(kc, P)],
                                Spair[:, t, :],
                                start=(t == 0),
                                stop=(t == NT - 1),
                            )
                        nc.vector.tensor_copy(xg2[:, kc, :], px[:])
                    _CACHE_XG[pair] = xg2
                xg = _CACHE_XG[pair][:, :, sub * C : (sub + 1) * C]

                # SW = S scaled by the per-token gate weight; STW = (S*w)^T
                SW = ht_pool.tile([P, NT, C], F16, tag="SW", name="SW")
                for t in range(NT):
                    nc.vector.tensor_scalar(
                        SW[:, t, :],
                        S[:, t, :],
                        Wt[:, t, e : e + 1],
                        Invt[:, t : t + 1],
                        ALU.mult,
                        ALU.mult,
                    )
                STW = sel_pool.tile([P, 2, TOK], F16, tag="STW", name="STW")
                for jj, (js, jn) in enumerate(JT):
                    for t in range(NT):
                        pst_full = psum_t.tile([P, P], F16, tag="pst", name="pst")
                        pst = pst_full[:jn, :]
                        nc.tensor.transpose(
                            pst[:], SW[:, t, js : js + jn], id128h[:]
                        )
                        nc.scalar.copy(STW[:jn, jj, ts(t, P)], pst[:])

                # hT[f, j] = silu(w1_e^T @ xg)
                ht = ht_pool.tile([P, KC_F, C], F16, tag="ht")
                for m in range(KC_F):
                    ph = psum_h.tile([P, FREE], F32, tag="ph", name="ph")[:, :C]
                    for kc in range(KC_D):
                        nc.tensor.matmul(
                            ph[:],
                            w1t[:, kc, ts(m, P)],
                            xg[:, kc, :],
                            start=(kc == 0),
                            stop=(kc == KC_D - 1),
                        )
                    nc.scalar.activation(ht[:, m, :], ph[:], AF.Silu)

                # y[j, d] = hT^T @ w2_e -> fp16 (weights applied via STW)
                ysb = y_pool.tile([P, 2, ND, FREE], F16, tag="ysb")
                for jj, (js, jn) in enumerate(JT):
                    for dh in range(ND):
                        py = psum_y.tile([P, FREE], F32, tag="py", name="py")[
                            :jn, :
                        ]
                        for k in range(KC_F):
                            nc.tensor.matmul(
                                py[:],
                                ht[:, k, js : js + jn],
                                w2t[:, k, ts(dh, FREE)],
                                start=(k == 0),
                                stop=(k == KC_F - 1),
                            )
                        nc.vector.tensor_copy(ysb[:jn, jj, dh, :], py[:])

                # scatter-add: acc[tok, d] += (S*w)^T^T... i.e. STW^T @ y
                for t in range(NT):
                    for dh in range(ND):
                        ps = psum_s.tile([P, FREE], F32, tag="pss", name="ps")
                        for jj, (js, jn) in enumerate(JT):
                            nc.tensor.matmul(
                                ps[:],
                                STW[:jn, jj, ts(t, P)],
                                ysb[:jn, jj, dh, :],
                                start=(jj == 0),
                                stop=(jj == len(JT) - 1),
                            )
                        nc.vector.tensor_add(
                            acc[:, t, ts(dh, FREE)], ps[:], acc[:, t, ts(dh, FREE)]
                        )
                        if e == E - 1:
                            nc.sync.dma_start(
                                out_r[:, t, ts(dh, FREE)], acc[:, t, ts(dh, FREE)]
                            )

    nc.compile()
    return nc


def kernel(norm_data, gate_w, w1, w2):
    global LAST_RESULT
    if "nc" not in _CACHE:
        _CACHE["nc"] = _build()
    nc = _CACHE["nc"]

    x = np.ascontiguousarray(np.asarray(norm_data, dtype=np.float32))
    gwT = np.ascontiguousarray(np.asarray(gate_w, dtype=np.float32).T)
    w1b = np.ascontiguousarray(np.asarray(w1, dtype=np.float32)).astype(np.float16)
    w2b = np.ascontiguousarray(np.asarray(w2, dtype=np.float32)).astype(np.float16)

    in_maps = []
    for c in range(N_CORES):
        xc = x[c * TOK : (c + 1) * TOK]
        in_maps.append(
            {
                "xT": np.ascontiguousarray(xc.T),
                "xh": xc.astype(np.float16),
                "gwT": gwT,
                "w1": w1b,
                "w2": w2b,
            }
        )

    trace = os.environ.get("KERNEL_TRACE", "0") == "1"
    if trace:
        import antenv

        ext = os.environ.get("KERNEL_TRACE_HOOK_DIR", "/root/antenv_ext")
        if ext not in antenv.__path__:
            antenv.__path__.append(ext)
        from antenv.axon_hooks import set_axon_ntff_profile_hook
        from trn_agent_boot.trn_boot import _ntff_profile_via_ctypes

        set_axon_ntff_profile_hook(
            _ntff_profile_via_ctypes("/opt/axon/libaxon_pjrt.so")
        )

    res = run_bass_kernel_spmd(
        nc, in_maps, core_ids=list(range(N_CORES)), trace=trace
    )
    LAST_RESULT = res

    out = np.concatenate(
        [res.results[c]["out"] for c in range(N_CORES)], axis=0
    )
    return out
